# revision 45
# baseline (speedup 1.0000x reference)
"""Causal multi-head attention (B=2, S=2048, D=1024, H=16) on 8 trn2 cores.

Sharding: core = (batch b = core//4, head-group g = core%4 of 4 heads).
Per core: Q/K/V projections for its 4 heads (Wq/Wk/Wv column-sharded),
causal attention, and the output projection against the row-shard of Wo.
The 4 per-batch partials are summed on the host (the TP all-reduce).

v3 layout (PSUM f32 everywhere):
  - Q/K/V projections run in fp8e4 + DoubleRow (cost: 0.5 cycles/row)
    with 3-term hi/lo error compensation (xh*wh + xh*wl + xl*wh).  The
    host pre-scales operands into fp8's normal range (x*4, W*16; the
    PSUM->SBUF copies divide by 64) and ships hi/lo splits pre-transposed
    as (D, S) fp8 pairs.  Attention and the output projection are bf16.
  - scores computed transposed, S^T (tk partitions, tq free), lhsT=K^T
    rhs=Q^T; head pairs occupy partitions 0-63 / 64-127.
  - P^T = exp(S^T/8) via one ACT op per (pair, tile) covering both heads
    (2-bank PSUM tile); causal masking = block skip + one shared [128,128]
    triangle mask multiplied in on GPSIMD, diagonal 128-col block only.
  - PV uses V (tokens, dk) + ones column so the softmax denominator
    accumulates free in PSUM row 64. out^T lands as (features, tokens),
    exactly the lhsT the output projection needs.
  - normalization: gather rowsums to partitions {0,32}, one reciprocal,
    pick-DMA + broadcast-DMA (sync queue) -> (128, tq); the multiply into
    a separate normalized oTn tile is DEFERRED one chunk so its DMA wait
    never blocks the in-order DVE queue.  The last chunk instead
    broadcasts via a [33,128] block-diagonal selector matmul on the PE
    (no DMA round-trip in the tail).
  - emission is software-pipelined: scores lead PV by 2 tiles;
    projection / output-projection units are metered in between attention
    tiles (carry-over ~500ns/tile budget) so the PE never waits for the
    saturated ACT exp; ~90 tiny warm-up matmuls cover the initial DMA
    fill (and ramp the clock-gate) before real work.
  HW quirks found: partition_broadcast ignores AP partition offsets
  (silently wrong), and Memset/TensorCopy/ISA ops reject partition
  offsets that are not multiples of 32 -- everything here sticks to
  offset-0/32/64 patterns proven in the earlier baseline.
"""

import numpy as np

B, S, D, H = 2, 2048, 1024, 16
DK = D // H               # 64
N_CORES = 8
G = 4                     # head-groups (cores per batch)
HPG = H // G              # 4 heads per core
NPAIR = HPG // 2          # 2 head-pairs per core
E = HPG * DK              # 256 per-core projection width
TQ = 512                  # tq chunk (PSUM bank width in f32)
NQ = S // TQ              # 4 tq chunks
TK = 128                  # tk tile
NK = S // TK              # 16 tk tiles
KD = 128                  # contraction tile over D
NKD = D // KD             # 8

_NC_CACHE = None


def _build():
    import concourse.bass as bass
    import concourse.tile as tile
    from concourse import bacc, mybir

    F32 = mybir.dt.float32
    BF16 = mybir.dt.bfloat16
    EXP = mybir.ActivationFunctionType.Exp

    nc = bacc.Bacc("TRN2", debug=False, num_devices=N_CORES)

    FP8 = mybir.dt.float8e4
    xq_d = [nc.dram_tensor(f"xqT{v}", (D, S), FP8, kind="ExternalInput").ap()
            for v in "hl"]
    xk_d = [nc.dram_tensor(f"xkT{v}", (D, S), FP8, kind="ExternalInput").ap()
            for v in "hl"]
    xv_d = [nc.dram_tensor(f"xvT{v}", (D, S), FP8, kind="ExternalInput").ap()
            for v in "hl"]
    wq_d = [nc.dram_tensor(f"wql{v}", (D, E), FP8, kind="ExternalInput").ap()
            for v in "hl"]
    wk_d = [nc.dram_tensor(f"wkl{v}", (D, E), FP8, kind="ExternalInput").ap()
            for v in "hl"]
    wv_d = [nc.dram_tensor(f"wvr{v}", (D, E), FP8, kind="ExternalInput").ap()
            for v in "hl"]
    wor = nc.dram_tensor("wor", (E, D), BF16, kind="ExternalInput").ap()
    mtri = nc.dram_tensor("mtri", (TK, TK), BF16, kind="ExternalInput").ap()
    seli = nc.dram_tensor("seli", (33, TK), BF16, kind="ExternalInput").ap()
    out = nc.dram_tensor("out", (S, D), BF16, kind="ExternalOutput").ap()

    with tile.TileContext(nc) as tc:
        with tc.tile_pool(name="consts", bufs=1) as consts, \
             tc.tile_pool(name="ppool", bufs=4) as ppool, \
             tc.tile_pool(name="norm", bufs=2) as norm, \
             tc.tile_pool(name="osb", bufs=4) as osb_pool, \
             tc.tile_pool(name="dr", bufs=2, space="DRAM") as dr, \
             tc.tile_pool(name="psum", bufs=1, space="PSUM") as psum:

            # ---- persistent SBUF ----
            wvr_sb = [consts.tile([128, NKD, E], FP8, name=f"wvr{v}")
                      for v in "hl"]
            wql_sb = [consts.tile([128, NKD, E], FP8, name=f"wql{v}")
                      for v in "hl"]
            wkl_sb = [consts.tile([128, NKD, E], FP8, name=f"wkl{v}")
                      for v in "hl"]
            wor_sb = consts.tile([128, NPAIR, D], BF16)
            mask_sb = consts.tile([128, TK], BF16)
            vaug = consts.tile([128, NK, HPG, DK + 1], BF16)
            # x activations fully resident, one tile per tq chunk
            xq_t = [[consts.tile([128, NKD, TQ], FP8, name=f"xq{c}{v}")
                     for c in range(NQ)] for v in range(2)]
            xk_t = [[consts.tile([128, NKD, TQ], FP8, name=f"xk{c}{v}")
                     for c in range(NQ)] for v in range(2)]
            xv_t = [[consts.tile([128, NKD, TQ], FP8, name=f"xv{c}{v}")
                     for c in range(NQ)] for v in range(2)]

            qT_sb = [consts.tile([128, S], BF16, name=f"qT{j}") for j in range(NPAIR)]
            kT_sb = [consts.tile([128, S], BF16, name=f"kT{j}") for j in range(NPAIR)]
            oT_sb = [consts.tile([128, S], BF16, name=f"oT{j}") for j in range(NPAIR)]
            oTn_sb = [consts.tile([128, S], BF16, name=f"oTn{j}") for j in range(NPAIR)]

            xq_r = [x.rearrange("(k p) t -> p k t", p=128) for x in xq_d]
            xk_r = [x.rearrange("(k p) t -> p k t", p=128) for x in xk_d]
            xv_r = [x.rearrange("(k p) t -> p k t", p=128) for x in xv_d]

            # ---- all input DMAs up front on the sync queue, in the order
            # compute consumes them; nothing here ever waits.  Transfers are
            # split <=0.5MB so the (serialized) DMA engine device never makes
            # a latecomer wait long. ----
            scratch = consts.tile([128, TK], BF16)
            nc.gpsimd.memset(scratch[:], 0.0)
            # [33,128] block-diagonal selector: broadcast-matmul for the
            # tail softmax normalization (row0 -> out parts 0-63, row32 ->
            # 64-127; rows 1-31 zero).  rcb is memset so its dead rows are
            # finite (0 * junk would otherwise poison the matmul).
            sel = consts.tile([33, TK], BF16)
            nc.sync.dma_start(sel[:], seli)
            rcb = consts.tile([33, TQ], BF16)
            nc.gpsimd.memset(rcb[:], 0.0)
            for v in range(2):
                nc.sync.dma_start(wql_sb[v][:],
                                  wq_d[v].rearrange("(k p) e -> p k e", p=128))
            for v in range(2):
                nc.sync.dma_start(xq_t[v][0][:], xq_r[v][:, :, 0:TQ])
            for v in range(2):
                nc.sync.dma_start(wkl_sb[v][:],
                                  wk_d[v].rearrange("(k p) e -> p k e", p=128))
            for v in range(2):
                nc.sync.dma_start(xk_t[v][0][:], xk_r[v][:, :, 0:TQ])
            for v in range(2):
                nc.sync.dma_start(wvr_sb[v][:],
                                  wv_d[v].rearrange("(k p) e -> p k e", p=128))
            for v in range(2):
                for q in range(2):
                    nc.sync.dma_start(xv_t[v][0][:, :, q * 256:(q + 1) * 256],
                                      xv_r[v][:, :, q * 256:(q + 1) * 256])
            nc.sync.dma_start(mask_sb[:], mtri.rearrange("p f -> p f"))
            for c in range(1, NQ):
                for t, t_r in ((xk_t, xk_r), (xq_t, xq_r), (xv_t, xv_r)):
                    for v in range(2):
                        sl = slice(c * TQ, (c + 1) * TQ)
                        nc.sync.dma_start(t[v][c][:], t_r[v][:, :, sl])
            nc.sync.dma_start(wor_sb[:], wor.rearrange("(j p) f -> p j f", p=128))

            # ---- PE warm-up: dummy matmuls on scratch while the first
            # loads land; ramps the clock gate so real work runs full speed.
            # The dummy result is parked in vaug's ones column (a read, to
            # satisfy the BIR verifier) and immediately memset to 1.0.
            dummy = psum.tile([128, TQ], F32, name="dummy", tag="pp", bufs=2)
            for _ in range(90):
                nc.tensor.matmul(dummy[:, 0:TK], scratch[:], scratch[:],
                                 start=True, stop=True)
            nc.vector.tensor_copy(vaug[:, 0, 0, DK:DK + 1], dummy[:, 0:1])
            # ones everywhere; V copies overwrite cols 0:DK of each head,
            # leaving column DK = 1.0 for the softmax denominator
            nc.gpsimd.memset(vaug[:], 1.0)

            # ---- emission units ----
            DR = mybir.MatmulPerfMode.DoubleRow
            TERMS = ((0, 0), (1, 0), (0, 1))  # (w variant, x variant); x-lo last

            def emit_v(m):
                # V projection for token tile m: fp8 DoubleRow, 3-term hi/lo
                c, part = divmod(m, 4)
                vp = psum.tile([128, E], F32, name=f"vp_{m}", tag="pp", bufs=2)
                nmm = 4 * len(TERMS)
                i = 0
                for kp in range(NKD // 2):
                    for wv, xv_ in TERMS:
                        nc.tensor.matmul(
                            vp[:],
                            xv_t[xv_][c][:, 2 * kp:2 * kp + 2,
                                         part * TK:(part + 1) * TK],
                            wvr_sb[wv][:, 2 * kp:2 * kp + 2, :],
                            start=(i == 0), stop=(i == nmm - 1),
                            perf_mode=DR,
                        )
                        i += 1
                nc.vector.tensor_scalar_mul(vaug[:, m, :, 0:DK], vp[:],
                                             1.0 / 64.0)

            def emit_qk(name, x_t, w_sb, dst, n, j, h=None):
                # Q^T / K^T projection: fp8 DoubleRow, 3-term hi/lo
                cols = slice(0, TQ) if h is None else slice(h * 256, (h + 1) * 256)
                w = cols.stop - cols.start
                pp = psum.tile([128, TQ], F32, name=f"pp_{name}_{n}_{j}_{h}",
                               tag="pp", bufs=2)
                nmm = 4 * len(TERMS)
                i = 0
                for kp in range(NKD // 2):
                    for wv, xv_ in TERMS:
                        nc.tensor.matmul(
                            pp[:, 0:w],
                            w_sb[wv][:, 2 * kp:2 * kp + 2, j * 128:(j + 1) * 128],
                            x_t[xv_][n][:, 2 * kp:2 * kp + 2, cols],
                            start=(i == 0), stop=(i == nmm - 1),
                            perf_mode=DR,
                        )
                        i += 1
                nc.vector.tensor_scalar_mul(
                    dst[j][:, n * TQ + cols.start:n * TQ + cols.stop],
                    pp[:, 0:w], 1.0 / 64.0)

            def emit_op(m, tail=False, ptag="pp"):
                # output projection for token tile m; store per half so the
                # last transfer after the final matmul is small.  In the tail
                # (post-attention) ACT is idle: give it the c=0 copy so the
                # two copies run in parallel.
                o_sb = osb_pool.tile([128, D], BF16, name=f"osb_{m}", tag="osb")
                for c in range(2):
                    op = psum.tile([128, TQ], F32, name=f"op_{m}_{c}",
                                   tag=(("pp" if c == 0 else "s2") if tail
                                        else ptag), bufs=2)
                    for j in range(NPAIR):
                        nc.tensor.matmul(
                            op[:],
                            oTn_sb[j][:, m * TK:(m + 1) * TK],
                            wor_sb[:, j, c * TQ:(c + 1) * TQ],
                            start=(j == 0), stop=(j == NPAIR - 1),
                        )
                    if tail and c == 0:
                        nc.scalar.copy(o_sb[:, c * TQ:(c + 1) * TQ], op[:])
                    elif tail and m == 4 * NQ - 1:
                        nc.scalar.copy(o_sb[:, c * TQ:c * TQ + 256],
                                       op[:, 0:256])
                        nc.vector.tensor_copy(
                            o_sb[:, c * TQ + 256:(c + 1) * TQ], op[:, 256:TQ])
                    else:
                        nc.vector.tensor_copy(o_sb[:, c * TQ:(c + 1) * TQ], op[:])
                    nc.sync.dma_start(
                        out[m * TK:(m + 1) * TK, c * TQ:(c + 1) * TQ],
                        o_sb[:, c * TQ:(c + 1) * TQ])

            # ---- filler machinery: pop queued projection/outproj units
            # between attention tiles so the PE never starves while ACT
            # works through the exp backlog ----
            fillers = []
            fill_acc = [0.0]

            def fill(budget_ns):
                fill_acc[0] += budget_ns
                while fillers and fillers[0][0] <= fill_acc[0]:
                    cost, emitfn = fillers.pop(0)
                    emitfn()
                    fill_acc[0] -= cost

            def drain():
                fill_acc[0] = 0.0
                while fillers:
                    fillers.pop(0)[1]()

            pending_mult = []

            def flush_mult():
                while pending_mult:
                    pending_mult.pop(0)()

            # ---- attention for one (pair, tq chunk), pipelined ----
            def emit_attention(j, n, deficit_ns, force_per_tile=0):
                flush_mult()
                n_tiles = 4 * n + 4
                pv = [
                    psum.tile([DK + 1, TQ], F32, name=f"pv_{j}_{n}_{hh}",
                              tag="pv", bufs=2)
                    for hh in range(2)
                ]
                p2s = [None] * n_tiles

                def emit_scores(i):
                    o = i - 4 * n
                    f0 = max(0, o * TK)
                    s2 = psum.tile([128, 2 * TQ], F32, name=f"s_{j}_{n}_{i}",
                                   tag="s2", bufs=2)
                    for hh in range(2):
                        nc.tensor.matmul(
                            s2[:, hh * TQ + f0:(hh + 1) * TQ],
                            kT_sb[j][hh * 64:(hh + 1) * 64, i * TK:(i + 1) * TK],
                            qT_sb[j][hh * 64:(hh + 1) * 64,
                                     n * TQ + f0:(n + 1) * TQ],
                            start=True, stop=True,
                        )
                    p2 = ppool.tile([128, 2 * TQ], BF16, name=f"p_{j}_{n}_{i}",
                                    tag="p")
                    p2s[i] = p2
                    if f0 == 0:
                        nc.scalar.activation(p2[:], s2[:], EXP, scale=0.125)
                    else:
                        w = TQ - f0
                        src = bass.AP(
                            tensor=s2.tensor, offset=s2[:, f0:].offset,
                            ap=[list(s2.ap[0]), [TQ, 2], [1, w]],
                        )
                        dst = bass.AP(
                            tensor=p2.tensor, offset=p2[:, f0:].offset,
                            ap=[list(p2.ap[0]), [TQ, 2], [1, w]],
                        )
                        nc.scalar.activation(dst, src, EXP, scale=0.125)
                    if o >= 0:
                        # causal triangle on the diagonal 128-col block only
                        for hh in range(2):
                            blk = p2[:, hh * TQ + f0:hh * TQ + f0 + TK]
                            nc.gpsimd.tensor_mul(blk, blk, mask_sb[:])

                def emit_pv(i):
                    o = i - 4 * n
                    f0 = max(0, o * TK)
                    p2 = p2s[i]
                    for hh in range(2):
                        nc.tensor.matmul(
                            pv[hh][:, f0:TQ],
                            vaug[:, i, 2 * j + hh, :],
                            p2[:, hh * TQ + f0:(hh + 1) * TQ],
                            start=(i == 0), stop=(i == n_tiles - 1),
                        )

                for i in range(n_tiles):
                    emit_scores(i)
                    if i >= 2:
                        emit_pv(i - 2)
                    for _ in range(force_per_tile):
                        if fillers:
                            fillers.pop(0)[1]()
                    fill(deficit_ns)
                emit_pv(n_tiles - 2)
                emit_pv(n_tiles - 1)

                # normalization (baseline-proven ops only): gather the two
                # PSUM rowsum rows to partitions {0,32}, one reciprocal,
                # pick-DMA to DRAM + broadcast-DMA back (sync queue); the
                # normalize multiply is DEFERRED one chunk so its bc2 wait
                # never blocks the in-order DVE queue.
                rs2 = norm.tile([33, TQ], F32, name=f"rs2_{j}_{n}", tag="rs2")
                for hh in range(2):
                    nc.scalar.copy(rs2[32 * hh:32 * hh + 1, :],
                                   pv[hh][DK:DK + 1, :])
                rc2 = norm.tile([33, TQ], F32, name=f"rc2_{j}_{n}", tag="rc2")
                nc.vector.reciprocal_approx_fast(rc2[:], rs2[:])
                for hh in range(2):
                    nc.vector.tensor_copy(
                        oT_sb[j][hh * 64:(hh + 1) * 64, n * TQ:(n + 1) * TQ],
                        pv[hh][0:DK, :])
                if (j, n) == (1, NQ - 1):
                    # tail: broadcast via one tiny PE matmul instead of the
                    # (slow round-trip) DMA bounce
                    nc.scalar.copy(rcb[0:1, :], rc2[0:1, :])
                    nc.vector.tensor_copy(rcb[32:33, :], rc2[32:33, :])

                    def multp(jj=j, nn=n, r=rcb):
                        bcp = psum.tile([128, TQ], F32, name="bcp", tag="pv",
                                        bufs=2)
                        nc.tensor.matmul(bcp[:], sel[:], r[:],
                                         start=True, stop=True)
                        for mm in range(4):
                            cs = slice(nn * TQ + mm * TK,
                                       nn * TQ + (mm + 1) * TK)
                            nc.vector.tensor_mul(
                                oTn_sb[jj][:, cs], oT_sb[jj][:, cs],
                                bcp[:, mm * TK:(mm + 1) * TK])
                    pending_mult.append(multp)
                    return
                rcd = dr.tile([2, TQ], F32, name=f"rcd_{j}_{n}", tag="rcd")
                nc.sync.dma_start(
                    rcd[:],
                    bass.AP(tensor=rc2.tensor, offset=rc2.offset,
                            ap=[[rc2.ap[0][0] * 32, 2], [1, TQ]]),
                )
                bc2 = norm.tile([128, TQ], F32, name=f"bc2_{j}_{n}", tag="bc2")
                nc.sync.dma_start(
                    bc2[:],
                    bass.AP(tensor=rcd.tensor, offset=rcd.offset,
                            ap=[[TQ, 2], [0, 64], [1, TQ]]),
                )

                def mult(jj=j, nn=n, b=bc2):
                    nc.vector.tensor_mul(
                        oTn_sb[jj][:, nn * TQ:(nn + 1) * TQ],
                        oT_sb[jj][:, nn * TQ:(nn + 1) * TQ], b[:])
                pending_mult.append(mult)

            # ---- prologue: Q/K for chunk 0 in half-units matching DMA
            # arrival order; V tiles 0-3 go in as forced fillers inside the
            # first attention chunk (their PV consumers come 2+ tiles in) ----
            for h in range(2):
                for j in range(NPAIR):
                    emit_qk("q", xq_t, wql_sb, qT_sb, 0, j, h)
            for h in range(2):
                for j in range(NPAIR):
                    emit_qk("k", xk_t, wkl_sb, kT_sb, 0, j, h)
            for m in range(4):
                fillers.append((642, (lambda mm: lambda: emit_v(mm))(m)))

            # per-tile PE filler budget (ns): ACT exp outpaces the 4 score+PV
            # matmuls of a tile by roughly this much once pipelined
            DEFICIT = [500, 500, 500, 500]

            for n in range(NQ):
                # fillers for chunk n+1 in DMA-arrival order (k, q, v);
                # outproj of chunk n-1 queues behind them (its deferred
                # normalize-mult flushes at the start of this chunk pair)
                if n + 1 < NQ:
                    for j in range(NPAIR):
                        fillers.append((1278, (lambda nn, jj: lambda: emit_qk(
                            "k", xk_t, wkl_sb, kT_sb, nn, jj))(n + 1, j)))
                    for j in range(NPAIR):
                        fillers.append((1278, (lambda nn, jj: lambda: emit_qk(
                            "q", xq_t, wql_sb, qT_sb, nn, jj))(n + 1, j)))
                    for m in range(4 * (n + 1), 4 * (n + 2)):
                        fillers.append((642, (lambda mm: lambda: emit_v(mm))(m)))
                emit_attention(0, n, DEFICIT[n],
                               force_per_tile=(1 if n == 0 else 0))
                emit_attention(1, n, DEFICIT[n])
                if n + 1 < NQ:
                    # everything chunk n+1 depends on must be in by now
                    drain()
                if n >= 1:
                    last = 4 * n - (2 if n == NQ - 1 else 0)
                    for m in range(4 * (n - 1), last):
                        fillers.append((852, (lambda mm: lambda: emit_op(mm))(m)))
            drain()
            # reserved units overlap the tail normalization chain
            emit_op(4 * (NQ - 1) - 2, tail=True)
            emit_op(4 * (NQ - 1) - 1, tail=True)
            flush_mult()
            for m in range(4 * (NQ - 1), 4 * NQ):
                emit_op(m, tail=True)

    nc.compile()
    return nc


def _get_nc():
    global _NC_CACHE
    if _NC_CACHE is None:
        _NC_CACHE = _build()
    return _NC_CACHE


def kernel(query, key, value, mask, Wq, Wk, Wv, Wo):
    import ml_dtypes
    from concourse.bass_utils import run_bass_kernel_spmd

    bf16 = ml_dtypes.bfloat16
    query = np.asarray(query, dtype=np.float32)
    key = np.asarray(key, dtype=np.float32)
    value = np.asarray(value, dtype=np.float32)
    mask = np.asarray(mask)
    Wq = np.asarray(Wq, dtype=np.float32)
    Wk = np.asarray(Wk, dtype=np.float32)
    Wv = np.asarray(Wv, dtype=np.float32)
    Wo = np.asarray(Wo, dtype=np.float32)

    # shared [128,128] causal triangle: keep iff local tq col f >= local tk
    # row p (diagonal-aligned blocks)
    mtri = np.ascontiguousarray(
        np.asarray(mask[0, :TK, :TK] != 0, dtype=np.float32).T).astype(bf16)
    seli_host = np.zeros((33, TK), np.float32)
    seli_host[0, 0:64] = 1.0
    seli_host[32, 64:128] = 1.0
    seli_host = seli_host.astype(bf16)

    f8 = ml_dtypes.float8_e4m3

    def hilo(a, scale):
        a = a * np.float32(scale)
        hi = a.astype(f8)
        lo = (a - hi.astype(np.float32)).astype(f8)
        return hi, lo

    xT = {}
    for b in range(B):
        xT[("q", b)] = hilo(np.ascontiguousarray(query[b].T), 4.0)
        xT[("k", b)] = hilo(np.ascontiguousarray(key[b].T), 4.0)
        xT[("v", b)] = hilo(np.ascontiguousarray(value[b].T), 4.0)

    in_maps = []
    for core in range(N_CORES):
        b, g = divmod(core, G)
        sl = slice(g * E, (g + 1) * E)
        wqh, wqlo = hilo(np.ascontiguousarray(Wq[sl, :].T), 16.0)
        wkh, wklo = hilo(np.ascontiguousarray(Wk[sl, :].T), 16.0)
        wvh, wvlo = hilo(np.ascontiguousarray(Wv[sl, :].T), 16.0)
        in_maps.append({
            "xqTh": xT[("q", b)][0], "xqTl": xT[("q", b)][1],
            "xkTh": xT[("k", b)][0], "xkTl": xT[("k", b)][1],
            "xvTh": xT[("v", b)][0], "xvTl": xT[("v", b)][1],
            "wqlh": wqh, "wqll": wqlo,
            "wklh": wkh, "wkll": wklo,
            "wvrh": wvh, "wvrl": wvlo,
            "wor": np.ascontiguousarray(Wo[:, sl].T).astype(bf16),
            "mtri": mtri,
            "seli": seli_host,
        })

    nc = _get_nc()
    res = run_bass_kernel_spmd(nc, in_maps, core_ids=list(range(N_CORES)))

    out = np.zeros((B, S, D), dtype=np.float32)
    for core in range(N_CORES):
        out[core // G] += np.asarray(res.results[core]["out"], dtype=np.float32)
    return out
